# revision 13
# baseline (speedup 1.0000x reference)
import sys

if '/opt/trn_rl_repo' not in sys.path:
    sys.path.insert(0, '/opt/trn_rl_repo')

import numpy as np
import ml_dtypes

import concourse.bacc as bacc
import concourse.mybir as mybir
from concourse.tile import TileContext
from concourse import bass_utils


def _install_queue_sem_lanes():
    """Pin each Pool SWDGE DMA's completion-sem lane to its queue_num.

    Tile's default round-robin lane assignment assumes FIFO completion
    across all SWDGE DMAs, which holds on one queue but breaks with
    multiple queues (out-of-order completion across queues lets a
    consumer's sem threshold be satisfied by a *different* DMA's
    increments). One lane per queue restores per-lane monotonicity."""
    import concourse.tile_sem_assignment as tsa
    if getattr(tsa.TileClockTick, '_qlane_patched', False):
        return
    orig = tsa.TileClockTick._assign_tick

    def patched(self, inst):
        qn = getattr(inst, 'queue_num', None)
        if (qn is not None and isinstance(inst, tsa.DMAInst)
                and inst.engine == mybir.EngineType.Pool):
            self.next_sw_dma_idx = int(qn) % self.swdge_sem_count
        return orig(self, inst)

    tsa.TileClockTick._assign_tick = patched
    tsa.TileClockTick._qlane_patched = True


def _install_ntff_shim():
    # Register the axon NTFF profile hook if the image's antenv lacks it.
    try:
        import antenv.axon_hooks  # noqa: F401
        return
    except ImportError:
        pass
    try:
        import types
        import trn_agent_boot.trn_boot as tb
        hook = tb._ntff_profile_via_ctypes('/opt/axon/libaxon_pjrt.so')
        if hook is None:
            return
        m = types.ModuleType('antenv.axon_hooks')
        m.get_axon_ntff_profile_hook = lambda: hook
        sys.modules['antenv.axon_hooks'] = m
        import antenv
        antenv.axon_hooks = m
        bass_utils.upload_artifacts = lambda d: "local://skipped"
    except Exception:
        pass

# ---------------- problem constants (hardcoded per spec) ----------------
N_NODES = 200000
D_IN = 64
D_OUT = 64
NUM_RELATIONS = 16
NUM_BASES = 8

N_CORES = 8
SHARD = 25000                 # nodes per core (target shard / source chunk)
SHARD_PAD = 25088             # 196*128
N_BLK = SHARD_PAD // 128      # 196 node blocks
N_GRP = N_BLK // 2            # 98 parity groups
HALF = SHARD_PAD // 2         # 12544
DUMP = 25080                  # scatter dump slot (inside padding)
WIN = 2048                    # scatter window (unique targets within)
TILE = 128                    # edges per matmul tile
GCALL_TILES = 32              # tiles per gather call (4096 idxs)
N_CHAINS = 4                  # accumulator chains
MAX_LEVELS = 12
R_ALL = NUM_RELATIONS + 1     # 17 (incl self row)

FP = mybir.dt.float32
BF = mybir.dt.bfloat16
I16 = mybir.dt.int16


# ---------------- host-side plan ----------------

def build_plan(source, target, edge_type):
    """Bucket directed edges by target shard; per core build a padded stream
    sorted by (level, src_chunk, rel). Eviction guarantees unique targets in
    every WIN-aligned window; evicted edges go to the next level. Group tile
    counts are uniform across cores (max-over-cores padding) so one SPMD
    program serves all cores."""
    src2 = np.concatenate([source, target]).astype(np.int64)
    tgt2 = np.concatenate([target, source]).astype(np.int64)
    et2 = np.concatenate([edge_type, edge_type]).astype(np.int64)

    core_of = tgt2 // SHARD
    carry = []
    for c in range(N_CORES):
        m = core_of == c
        s = src2[m]
        carry.append({
            'chunk': (s // SHARD).astype(np.int64),
            'sloc': (s % SHARD).astype(np.int64),
            'rel': et2[m],
            'tloc': (tgt2[m] % SHARD).astype(np.int64),
        })

    levels = []
    gparts = [[] for _ in range(N_CORES)]
    sparts = [[] for _ in range(N_CORES)]
    pos = 0

    for _lv in range(MAX_LEVELS):
        if all(len(e['rel']) == 0 for e in carry):
            break
        orders = []
        for c in range(N_CORES):
            e = carry[c]
            o = np.lexsort((e['rel'], e['chunk']))
            orders.append({k: v[o] for k, v in e.items()})
        # uniform per-group tile counts (pre-eviction upper bound)
        tiles = np.zeros((N_CORES, N_CORES, NUM_RELATIONS), np.int64)
        for c in range(N_CORES):
            e = orders[c]
            key = e['chunk'] * NUM_RELATIONS + e['rel']
            cnt = np.bincount(key, minlength=N_CORES * NUM_RELATIONS)
            tiles[c] = -(-cnt.reshape(N_CORES, NUM_RELATIONS) // TILE)
        ut = tiles.max(axis=0)
        levels.append(ut)

        next_carry = []
        end_pos = pos + int(ut.sum()) * TILE
        for c in range(N_CORES):
            e = orders[c]
            key = e['chunk'] * NUM_RELATIONS + e['rel']
            cnt = np.bincount(key, minlength=N_CORES * NUM_RELATIONS)
            starts = np.concatenate([[0], np.cumsum(cnt)])
            gbuf, sbuf_ = [], []
            ev = {k: [] for k in ('chunk', 'sloc', 'rel', 'tloc')}
            win_seen = {}
            p = pos
            sloc, tloc, chk = e['sloc'], e['tloc'], e['chunk']
            for ch in range(N_CORES):
                for r in range(NUM_RELATIONS):
                    gi = ch * NUM_RELATIONS + r
                    a, b = int(starts[gi]), int(starts[gi + 1])
                    cap = int(ut[ch][r]) * TILE
                    kept = 0
                    for j in range(a, b):
                        t = int(tloc[j])
                        w = p // WIN
                        ws = win_seen.get(w)
                        if ws is None:
                            ws = win_seen[w] = set()
                        if t in ws:
                            ev['chunk'].append(int(chk[j]))
                            ev['sloc'].append(int(sloc[j]))
                            ev['rel'].append(r)
                            ev['tloc'].append(t)
                            continue
                        ws.add(t)
                        gbuf.append(int(sloc[j]))
                        sbuf_.append(t)
                        kept += 1
                        p += 1
                    npad = cap - kept
                    if npad:
                        gbuf.extend([0] * npad)
                        sbuf_.extend([DUMP] * npad)
                        p += npad
            assert p == end_pos
            gparts[c].append(np.asarray(gbuf, np.int16))
            sparts[c].append(np.asarray(sbuf_, np.int16))
            next_carry.append({k: np.asarray(v, np.int64) for k, v in ev.items()})
        pos = end_pos
        carry = next_carry
    else:
        raise RuntimeError("eviction did not converge; raise MAX_LEVELS")

    S = pos
    S_pad = -(-S // WIN) * WIN
    extra = S_pad - S
    if extra:
        assert extra % TILE == 0
        levels[-1][N_CORES - 1][NUM_RELATIONS - 1] += extra // TILE
        for c in range(N_CORES):
            gparts[c].append(np.zeros(extra, np.int16))
            sparts[c].append(np.full(extra, DUMP, np.int16))

    gidx_all = np.stack([np.concatenate(g) for g in gparts])
    sidx_all = np.stack([np.concatenate(s) for s in sparts])
    assert gidx_all.shape == (N_CORES, S_pad) and sidx_all.shape == (N_CORES, S_pad)
    return levels, gidx_all, sidx_all


def wrap16(arr2d):
    """[C, S] -> [C, 128, S//16] int16 in the Q7 wrapped+replicated layout."""
    C, S = arr2d.shape
    w = arr2d.reshape(C, S // 16, 16).transpose(0, 2, 1)
    return np.ascontiguousarray(np.tile(w, (1, 8, 1))).astype(np.int16)


# ---------------- device program ----------------

def build_nc(levels, S):
    import os
    EN_SCATTER = os.environ.get('K_NO_SCATTER', '0') != '1'
    EN_GATHER = os.environ.get('K_NO_GATHER', '0') != '1'
    MAX_GCALLS = int(os.environ.get('K_MAX_GCALLS', '1000000'))
    EN_SELF = os.environ.get('K_NO_SELF', '0') != '1'
    NQ = int(os.environ.get('K_QUEUES', '1'))
    QSPLIT = os.environ.get('K_QSPLIT', '0') == '1'
    SCRATCH = int(os.environ.get('K_SCRATCH', '16384'))
    if NQ > 1:
        _install_queue_sem_lanes()
    nc = bacc.Bacc("TRN2", debug=False, num_swdge_queues=NQ,
                   dynamic_dma_scratch_size=SCRATCH)

    x_d = nc.dram_tensor("x", [N_NODES, 128], BF, kind="ExternalInput")
    xt_d = nc.dram_tensor("xt", [128, HALF], BF, kind="ExternalInput")
    mask_d = nc.dram_tensor("mask", [128, N_BLK], FP, kind="ExternalInput")
    attT_d = nc.dram_tensor("attT", [NUM_BASES, R_ALL], FP, kind="ExternalInput")
    bas_d = nc.dram_tensor("bas", [NUM_BASES, D_IN * D_OUT], FP, kind="ExternalInput")
    gi_d = nc.dram_tensor("gi", [128, S // 16], I16, kind="ExternalInput")
    si_d = nc.dram_tensor("si", [128, S // 16], I16, kind="ExternalInput")
    w_stage_d = nc.dram_tensor("wstage", [R_ALL, D_IN * D_OUT], BF, kind="Internal")
    oute_d = nc.dram_tensor("oute", [128, N_GRP, D_OUT], FP, kind="ExternalOutput")
    outo_d = nc.dram_tensor("outo", [128, N_GRP, D_OUT], FP, kind="ExternalOutput")

    # flat per-tile (level, chunk, rel)
    tile_meta = []
    for ut in levels:
        for ch in range(N_CORES):
            for r in range(NUM_RELATIONS):
                tile_meta.extend([(id(ut), ch, r)] * int(ut[ch][r]))
    assert len(tile_meta) * TILE == S

    # gather calls: contiguous same-(level,chunk) runs of <= GCALL_TILES tiles
    gcalls = []
    i = 0
    while i < len(tile_meta):
        lv, ch, _ = tile_meta[i]
        j = i
        while (j < len(tile_meta) and j - i < GCALL_TILES
               and tile_meta[j][0] == lv and tile_meta[j][1] == ch):
            j += 1
        gcalls.append((i, j - i, ch))
        i = j

    XBUF = int(os.environ.get('K_XBUF', '0'))
    with TileContext(nc) as tc:
        with (
            tc.tile_pool(name="const", bufs=1) as constp,
            tc.tile_pool(name="gpool", bufs=2) as gpool,
            tc.tile_pool(name="mpool", bufs=6 + 4 * XBUF) as mpool,
            tc.tile_pool(name="gip", bufs=3) as gip,
            tc.tile_pool(name="sip", bufs=4 + 4 * XBUF) as sip,
            tc.tile_pool(name="stp", bufs=(1 if SCRATCH > 16384 else 2)) as stp,
            tc.tile_pool(name="wps", bufs=1, space="PSUM") as wps,
            tc.tile_pool(name="sps", bufs=3, space="PSUM") as sps,
            tc.tile_pool(name="mps", bufs=4, space="PSUM") as mps,
        ):
            attT = constp.tile([NUM_BASES, R_ALL], FP)
            bas = constp.tile([NUM_BASES, D_IN * D_OUT], FP)
            w_all = constp.tile([128, R_ALL, D_OUT], BF)
            w_stage = constp.tile([R_ALL, D_IN * D_OUT], BF)
            xt_sb = constp.tile([128, HALF], BF)
            mask_sb = constp.tile([128, N_BLK], FP)
            accs = [(constp.tile([128, N_GRP, D_OUT], BF, name=f"a{k}e"),
                     constp.tile([128, N_GRP, D_OUT], BF, name=f"a{k}o"))
                    for k in range(N_CHAINS)]

            nc.sync.dma_start(attT[:], attT_d[:])
            nc.sync.dma_start(bas[:], bas_d[:])
            nc.sync.dma_start(xt_sb[:], xt_d[:])
            nc.sync.dma_start(mask_sb[:], mask_d[:])
            for k in range((1 if EN_SELF else 0), N_CHAINS):
                nc.gpsimd.memset(accs[k][0][:], 0.0)
                nc.gpsimd.memset(accs[k][1][:], 0.0)

            # ---- W = attT.T @ bases (staged through DRAM to transpose) ----
            for j in range(8):
                wp = wps.tile([R_ALL, 512], FP)
                nc.tensor.matmul(wp[:], attT[:], bas[:, j * 512:(j + 1) * 512],
                                 start=True, stop=True)
                nc.scalar.copy(w_stage[:, j * 512:(j + 1) * 512], wp[:])
            nc.sync.dma_start(w_stage_d[:], w_stage[:])
            w_re = w_stage_d.rearrange("r (d o) -> d r o", d=D_IN, o=D_OUT)
            nc.sync.dma_start(w_all[0:64, :, :], w_re)
            nc.sync.dma_start(w_all[64:128, :, :], w_re)

            # ---- self-loop term into chain-0 accumulators ----
            for b in range(N_BLK if EN_SELF else 0):
                half = (b * 128) // HALF
                col = (b * 128) % HALF
                sp = sps.tile([128, D_OUT], FP)
                nc.tensor.matmul(
                    sp[:], xt_sb[64 * half:64 * half + 64, col:col + 128],
                    w_all[64 * half:64 * half + 64, NUM_RELATIONS, :],
                    start=True, stop=True)
                dst = accs[0][b % 2]
                nc.scalar.activation(
                    dst[:, b // 2, :], sp[:],
                    mybir.ActivationFunctionType.Copy,
                    scale=mask_sb[:, b:b + 1])

            # ---- main pipeline ----
            win_m = {}
            banks = {}
            for gci, (t0, ntl, ch) in enumerate(gcalls[:MAX_GCALLS]):
                nidx = ntl * TILE
                gt = gpool.tile([128, 1, GCALL_TILES * TILE], BF, tag="g")
                gi_sb = gip.tile([128, GCALL_TILES * TILE // 16], I16, tag="gi")
                nc.sync.dma_start(
                    gi_sb[:, :nidx // 16],
                    gi_d[:, t0 * TILE // 16:(t0 * TILE + nidx) // 16])
                if EN_GATHER: nc.gpsimd.dma_gather(
                    gt[:, :, :nidx],
                    x_d[ch * SHARD:(ch + 1) * SHARD, :],
                    gi_sb[:, :nidx // 16],
                    nidx, nidx, 128, elem_step=128, transpose=True,
                    single_packet=False,
                    queue_num=(1 if QSPLIT else gci % NQ),
                )
                for tt in range(ntl):
                    t = t0 + tt
                    r = tile_meta[t][2]
                    w = t // 16
                    sl = t % 16
                    if sl == 0:
                        win_m[w] = mpool.tile([128, 16 * D_OUT], BF, tag="m", name=f"m{w}")
                    if t % 8 == 0:
                        banks[t] = mps.tile([128, 512], FP, tag="bank", name=f"bank{t}")
                    bank = banks[t - t % 8]
                    nc.tensor.matmul(
                        bank[:, (t % 8) * 64:(t % 8) * 64 + 64],
                        gt[0:D_IN, 0, tt * TILE:(tt + 1) * TILE],
                        w_all[0:64, r, :],
                        start=True, stop=True)
                    if t % 8 == 7:
                        h = (sl // 8)
                        dst = win_m[w][:, h * 512:(h + 1) * 512]
                        if (t // 8) % 2 == 0:
                            nc.scalar.copy(dst, bank[:])
                        else:
                            nc.vector.tensor_copy(dst, bank[:])
                        del banks[t - 7]
                    if sl == 15:
                        si_sb = sip.tile([128, WIN // 16], I16, tag="si")
                        nc.sync.dma_start(
                            si_sb[:], si_d[:, w * WIN // 16:(w + 1) * WIN // 16])
                        ae, ao = accs[w % N_CHAINS]
                        if EN_SCATTER: nc.gpsimd.dma_scatter_add(
                            ae[:],
                            win_m[w][:].rearrange("p (b e) -> p b e", e=D_OUT),
                            si_sb[:], WIN, WIN, D_OUT,
                            sbuf_tokens_per_rank=128, parity_reg=0,
                            out_ap_other=ao[:],
                            single_packet=os.environ.get('K_SP', '0') == '1',
                            queue_num=(0 if QSPLIT else w % NQ),
                        )
                        del win_m[w]

            # ---- combine chains and write out ----
            CH = 14  # 98 = 7*14
            for par, out_d in ((0, oute_d), (1, outo_d)):
                for g0 in range(0, N_GRP, CH):
                    st = stp.tile([128, CH, D_OUT], FP, tag="st")
                    st2 = stp.tile([128, CH, D_OUT], FP, tag="st2")
                    sls = (slice(None), slice(g0, g0 + CH), slice(None))
                    nc.vector.tensor_add(st[:], accs[0][par][sls], accs[1][par][sls])
                    nc.vector.tensor_add(st2[:], accs[2][par][sls], accs[3][par][sls])
                    nc.vector.tensor_add(st[:], st[:], st2[:])
                    nc.sync.dma_start(out_d[:, g0:g0 + CH, :], st[:])

    nc.compile()
    return nc


# ---------------- top-level kernel ----------------

def kernel(x, node_keep_mask, source, target, edge_type, bases, att):
    x = np.asarray(x, np.float32)
    mask = np.asarray(node_keep_mask)
    bases = np.asarray(bases, np.float32)
    att = np.asarray(att, np.float32)

    import os
    import hashlib
    levels = gidx_all = sidx_all = None
    _h = hashlib.sha1()
    for _a in (source, target, edge_type):
        _h.update(np.ascontiguousarray(np.asarray(_a)).tobytes())
    _pc = f'/tmp/rgcn_plan_v1_{_h.hexdigest()[:12]}.npz'
    if os.environ.get('K_PLAN_CACHE', '1') == '1' and os.path.exists(_pc):
        try:
            _d = np.load(_pc)
            levels = [lv for lv in _d['levels']]
            gidx_all, sidx_all = _d['gidx'], _d['sidx']
        except Exception:
            levels = None
    if levels is None:
        levels, gidx_all, sidx_all = build_plan(
            np.asarray(source), np.asarray(target), np.asarray(edge_type))
        try:
            np.savez(_pc, levels=np.stack(levels), gidx=gidx_all, sidx=sidx_all)
        except Exception:
            pass
    S = gidx_all.shape[1]
    nc = build_nc(levels, S)

    x_pad = np.zeros((N_NODES, 128), ml_dtypes.bfloat16)
    x_pad[:, :D_IN] = x.astype(ml_dtypes.bfloat16)
    attT = np.ascontiguousarray(att.T)
    bas = np.ascontiguousarray(bases.reshape(NUM_BASES, -1))
    gi_w = wrap16(gidx_all)
    si_w = wrap16(sidx_all)

    in_maps = []
    for c in range(N_CORES):
        xs = np.zeros((SHARD_PAD, D_IN), np.float32)
        xs[:SHARD] = x[c * SHARD:(c + 1) * SHARD]
        xt = xs.T.astype(ml_dtypes.bfloat16)                    # [64, 25088]
        xt128 = np.ascontiguousarray(
            np.concatenate([xt[:, :HALF], xt[:, HALF:]], axis=0))  # [128, 12544]
        mk = np.zeros(SHARD_PAD, np.float32)
        mk[:SHARD] = mask[c * SHARD:(c + 1) * SHARD].astype(np.float32)
        mk = np.ascontiguousarray(mk.reshape(N_BLK, 128).T)     # [128, 196]
        in_maps.append({
            "x": x_pad, "xt": xt128, "mask": mk, "attT": attT, "bas": bas,
            "gi": gi_w[c], "si": si_w[c],
        })

    import os
    trace = os.environ.get("K_TRACE", "0") == "1"
    if trace:
        _install_ntff_shim()
    res = bass_utils.run_bass_kernel_spmd(
        nc, in_maps, core_ids=list(range(N_CORES)), trace=trace)
    if trace and res.exec_time_ns is not None:
        print(f"HW exec time: {res.exec_time_ns} ns", flush=True)
        kernel.last_exec_time_ns = res.exec_time_ns

    out = np.zeros((N_NODES, D_OUT), np.float32)
    v = np.arange(SHARD)
    sl, pt = v // 128, v % 128
    ev = (sl % 2) == 0
    for c in range(N_CORES):
        oe = res.results[c]["oute"]
        oo = res.results[c]["outo"]
        out[c * SHARD:(c + 1) * SHARD] = np.where(
            ev[:, None], oe[pt, sl // 2, :], oo[pt, sl // 2, :])
    return out



# revision 15
# speedup vs baseline: 1.0087x; 1.0087x over previous
import sys

if '/opt/trn_rl_repo' not in sys.path:
    sys.path.insert(0, '/opt/trn_rl_repo')

import numpy as np
import ml_dtypes

import concourse.bacc as bacc
import concourse.mybir as mybir
from concourse.tile import TileContext
from concourse import bass_utils


def _install_queue_sem_lanes():
    """Pin each Pool SWDGE DMA's completion-sem lane to its queue_num.

    Tile's default round-robin lane assignment assumes FIFO completion
    across all SWDGE DMAs, which holds on one queue but breaks with
    multiple queues (out-of-order completion across queues lets a
    consumer's sem threshold be satisfied by a *different* DMA's
    increments). One lane per queue restores per-lane monotonicity."""
    import concourse.tile_sem_assignment as tsa
    if getattr(tsa.TileClockTick, '_qlane_patched', False):
        return
    orig = tsa.TileClockTick._assign_tick

    def patched(self, inst):
        qn = getattr(inst, 'queue_num', None)
        if (qn is not None and isinstance(inst, tsa.DMAInst)
                and inst.engine == mybir.EngineType.Pool):
            self.next_sw_dma_idx = int(qn) % self.swdge_sem_count
        return orig(self, inst)

    tsa.TileClockTick._assign_tick = patched
    tsa.TileClockTick._qlane_patched = True


def _install_ntff_shim():
    # Register the axon NTFF profile hook if the image's antenv lacks it.
    try:
        import antenv.axon_hooks  # noqa: F401
        return
    except ImportError:
        pass
    try:
        import types
        import trn_agent_boot.trn_boot as tb
        hook = tb._ntff_profile_via_ctypes('/opt/axon/libaxon_pjrt.so')
        if hook is None:
            return
        m = types.ModuleType('antenv.axon_hooks')
        m.get_axon_ntff_profile_hook = lambda: hook
        sys.modules['antenv.axon_hooks'] = m
        import antenv
        antenv.axon_hooks = m
        bass_utils.upload_artifacts = lambda d: "local://skipped"
    except Exception:
        pass

# ---------------- problem constants (hardcoded per spec) ----------------
N_NODES = 200000
D_IN = 64
D_OUT = 64
NUM_RELATIONS = 16
NUM_BASES = 8

N_CORES = 8
SHARD = 25000                 # nodes per core (target shard / source chunk)
SHARD_PAD = 25088             # 196*128
N_BLK = SHARD_PAD // 128      # 196 node blocks
N_GRP = N_BLK // 2            # 98 parity groups
HALF = SHARD_PAD // 2         # 12544
DUMP = 25080                  # scatter dump slot (inside padding)
WIN = 2048                    # scatter window (unique targets within)
TILE = 128                    # edges per matmul tile
GCALL_TILES = 32              # tiles per gather call (4096 idxs)
N_CHAINS = 4                  # accumulator chains
MAX_LEVELS = 12
R_ALL = NUM_RELATIONS + 1     # 17 (incl self row)

FP = mybir.dt.float32
BF = mybir.dt.bfloat16
I16 = mybir.dt.int16


# ---------------- host-side plan ----------------

def build_plan(source, target, edge_type):
    """Bucket directed edges by target shard; per core build a padded stream
    sorted by (level, src_chunk, rel). Eviction guarantees unique targets in
    every WIN-aligned window; evicted edges go to the next level. Group tile
    counts are uniform across cores (max-over-cores padding) so one SPMD
    program serves all cores."""
    src2 = np.concatenate([source, target]).astype(np.int64)
    tgt2 = np.concatenate([target, source]).astype(np.int64)
    et2 = np.concatenate([edge_type, edge_type]).astype(np.int64)

    core_of = tgt2 // SHARD
    carry = []
    for c in range(N_CORES):
        m = core_of == c
        s = src2[m]
        carry.append({
            'chunk': (s // SHARD).astype(np.int64),
            'sloc': (s % SHARD).astype(np.int64),
            'rel': et2[m],
            'tloc': (tgt2[m] % SHARD).astype(np.int64),
        })

    levels = []
    gparts = [[] for _ in range(N_CORES)]
    sparts = [[] for _ in range(N_CORES)]
    pos = 0

    for _lv in range(MAX_LEVELS):
        if all(len(e['rel']) == 0 for e in carry):
            break
        orders = []
        for c in range(N_CORES):
            e = carry[c]
            o = np.lexsort((e['rel'], e['chunk']))
            orders.append({k: v[o] for k, v in e.items()})
        # uniform per-group tile counts (pre-eviction upper bound)
        tiles = np.zeros((N_CORES, N_CORES, NUM_RELATIONS), np.int64)
        for c in range(N_CORES):
            e = orders[c]
            key = e['chunk'] * NUM_RELATIONS + e['rel']
            cnt = np.bincount(key, minlength=N_CORES * NUM_RELATIONS)
            tiles[c] = -(-cnt.reshape(N_CORES, NUM_RELATIONS) // TILE)
        ut = tiles.max(axis=0)
        levels.append(ut)

        next_carry = []
        end_pos = pos + int(ut.sum()) * TILE
        for c in range(N_CORES):
            e = orders[c]
            key = e['chunk'] * NUM_RELATIONS + e['rel']
            cnt = np.bincount(key, minlength=N_CORES * NUM_RELATIONS)
            starts = np.concatenate([[0], np.cumsum(cnt)])
            gbuf, sbuf_ = [], []
            ev = {k: [] for k in ('chunk', 'sloc', 'rel', 'tloc')}
            win_seen = {}
            p = pos
            sloc, tloc, chk = e['sloc'], e['tloc'], e['chunk']
            for ch in range(N_CORES):
                for r in range(NUM_RELATIONS):
                    gi = ch * NUM_RELATIONS + r
                    a, b = int(starts[gi]), int(starts[gi + 1])
                    cap = int(ut[ch][r]) * TILE
                    kept = 0
                    for j in range(a, b):
                        t = int(tloc[j])
                        w = p // WIN
                        ws = win_seen.get(w)
                        if ws is None:
                            ws = win_seen[w] = set()
                        if t in ws:
                            ev['chunk'].append(int(chk[j]))
                            ev['sloc'].append(int(sloc[j]))
                            ev['rel'].append(r)
                            ev['tloc'].append(t)
                            continue
                        ws.add(t)
                        gbuf.append(int(sloc[j]))
                        sbuf_.append(t)
                        kept += 1
                        p += 1
                    npad = cap - kept
                    if npad:
                        gbuf.extend([0] * npad)
                        sbuf_.extend([DUMP] * npad)
                        p += npad
            assert p == end_pos
            gparts[c].append(np.asarray(gbuf, np.int16))
            sparts[c].append(np.asarray(sbuf_, np.int16))
            next_carry.append({k: np.asarray(v, np.int64) for k, v in ev.items()})
        pos = end_pos
        carry = next_carry
    else:
        raise RuntimeError("eviction did not converge; raise MAX_LEVELS")

    S = pos
    S_pad = -(-S // WIN) * WIN
    extra = S_pad - S
    if extra:
        assert extra % TILE == 0
        levels[-1][N_CORES - 1][NUM_RELATIONS - 1] += extra // TILE
        for c in range(N_CORES):
            gparts[c].append(np.zeros(extra, np.int16))
            sparts[c].append(np.full(extra, DUMP, np.int16))

    gidx_all = np.stack([np.concatenate(g) for g in gparts])
    sidx_all = np.stack([np.concatenate(s) for s in sparts])
    assert gidx_all.shape == (N_CORES, S_pad) and sidx_all.shape == (N_CORES, S_pad)
    return levels, gidx_all, sidx_all


def wrap16(arr2d):
    """[C, S] -> [C, 128, S//16] int16 in the Q7 wrapped+replicated layout."""
    C, S = arr2d.shape
    w = arr2d.reshape(C, S // 16, 16).transpose(0, 2, 1)
    return np.ascontiguousarray(np.tile(w, (1, 8, 1))).astype(np.int16)


# ---------------- device program ----------------

def build_nc(levels, S):
    import os
    EN_SCATTER = os.environ.get('K_NO_SCATTER', '0') != '1'
    EN_GATHER = os.environ.get('K_NO_GATHER', '0') != '1'
    MAX_GCALLS = int(os.environ.get('K_MAX_GCALLS', '1000000'))
    EN_SELF = os.environ.get('K_NO_SELF', '0') != '1'
    NQ = int(os.environ.get('K_QUEUES', '1'))
    QSPLIT = os.environ.get('K_QSPLIT', '0') == '1'
    SCRATCH = int(os.environ.get('K_SCRATCH', '16384'))
    if NQ > 1:
        _install_queue_sem_lanes()
    nc = bacc.Bacc("TRN2", debug=False, num_swdge_queues=NQ,
                   dynamic_dma_scratch_size=SCRATCH)

    x_d = nc.dram_tensor("x", [N_NODES, 128], BF, kind="ExternalInput")
    xt_d = nc.dram_tensor("xt", [128, HALF], BF, kind="ExternalInput")
    mask_d = nc.dram_tensor("mask", [128, N_BLK], FP, kind="ExternalInput")
    attT_d = nc.dram_tensor("attT", [NUM_BASES, R_ALL], FP, kind="ExternalInput")
    bas_d = nc.dram_tensor("bas", [NUM_BASES, D_IN * D_OUT], FP, kind="ExternalInput")
    gi_d = nc.dram_tensor("gi", [128, S // 16], I16, kind="ExternalInput")
    si_d = nc.dram_tensor("si", [128, S // 16], I16, kind="ExternalInput")
    w_stage_d = nc.dram_tensor("wstage", [R_ALL, D_IN * D_OUT], BF, kind="Internal")
    oute_d = nc.dram_tensor("oute", [128, N_GRP, D_OUT], FP, kind="ExternalOutput")
    outo_d = nc.dram_tensor("outo", [128, N_GRP, D_OUT], FP, kind="ExternalOutput")

    # flat per-tile (level, chunk, rel)
    tile_meta = []
    for ut in levels:
        for ch in range(N_CORES):
            for r in range(NUM_RELATIONS):
                tile_meta.extend([(id(ut), ch, r)] * int(ut[ch][r]))
    assert len(tile_meta) * TILE == S

    # gather calls: contiguous same-(level,chunk) runs of <= GCALL_TILES tiles
    gcalls = []
    i = 0
    while i < len(tile_meta):
        lv, ch, _ = tile_meta[i]
        j = i
        while (j < len(tile_meta) and j - i < GCALL_TILES
               and tile_meta[j][0] == lv and tile_meta[j][1] == ch):
            j += 1
        gcalls.append((i, j - i, ch))
        i = j

    XBUF = int(os.environ.get('K_XBUF', '0'))
    with TileContext(nc) as tc:
        with (
            tc.tile_pool(name="const", bufs=1) as constp,
            tc.tile_pool(name="gpool", bufs=2) as gpool,
            tc.tile_pool(name="mpool", bufs=6 + 4 * XBUF) as mpool,
            tc.tile_pool(name="gip", bufs=3) as gip,
            tc.tile_pool(name="sip", bufs=4 + 4 * XBUF) as sip,
            tc.tile_pool(name="stp", bufs=(1 if SCRATCH > 16384 else 2)) as stp,
            tc.tile_pool(name="wps", bufs=1, space="PSUM") as wps,
            tc.tile_pool(name="sps", bufs=3, space="PSUM") as sps,
            tc.tile_pool(name="mps", bufs=4, space="PSUM") as mps,
        ):
            attT = constp.tile([NUM_BASES, R_ALL], FP)
            bas = constp.tile([NUM_BASES, D_IN * D_OUT], FP)
            w_all = constp.tile([128, R_ALL, D_OUT], BF)
            w_stage = constp.tile([R_ALL, D_IN * D_OUT], BF)
            xt_sb = constp.tile([128, HALF], BF)
            mask_sb = constp.tile([128, N_BLK], FP)
            accs = [(constp.tile([128, N_GRP, D_OUT], BF, name=f"a{k}e"),
                     constp.tile([128, N_GRP, D_OUT], BF, name=f"a{k}o"))
                    for k in range(N_CHAINS)]

            nc.sync.dma_start(attT[:], attT_d[:])
            nc.sync.dma_start(bas[:], bas_d[:])
            nc.sync.dma_start(xt_sb[:], xt_d[:])
            nc.sync.dma_start(mask_sb[:], mask_d[:])
            for k in range((1 if EN_SELF else 0), N_CHAINS):
                nc.vector.memset(accs[k][0][:], 0.0)
                nc.vector.memset(accs[k][1][:], 0.0)

            # ---- W = attT.T @ bases (staged through DRAM to transpose) ----
            for j in range(8):
                wp = wps.tile([R_ALL, 512], FP)
                nc.tensor.matmul(wp[:], attT[:], bas[:, j * 512:(j + 1) * 512],
                                 start=True, stop=True)
                nc.scalar.copy(w_stage[:, j * 512:(j + 1) * 512], wp[:])
            nc.sync.dma_start(w_stage_d[:], w_stage[:])
            w_re = w_stage_d.rearrange("r (d o) -> d r o", d=D_IN, o=D_OUT)
            nc.sync.dma_start(w_all[0:64, :, :], w_re)
            nc.sync.dma_start(w_all[64:128, :, :], w_re)

            # ---- self-loop term into chain-0 accumulators ----
            for b in range(N_BLK if EN_SELF else 0):
                half = (b * 128) // HALF
                col = (b * 128) % HALF
                sp = sps.tile([128, D_OUT], FP)
                nc.tensor.matmul(
                    sp[:], xt_sb[64 * half:64 * half + 64, col:col + 128],
                    w_all[64 * half:64 * half + 64, NUM_RELATIONS, :],
                    start=True, stop=True)
                dst = accs[0][b % 2]
                nc.scalar.activation(
                    dst[:, b // 2, :], sp[:],
                    mybir.ActivationFunctionType.Copy,
                    scale=mask_sb[:, b:b + 1])

            # ---- main pipeline ----
            win_m = {}
            banks = {}
            for gci, (t0, ntl, ch) in enumerate(gcalls[:MAX_GCALLS]):
                nidx = ntl * TILE
                gt = gpool.tile([128, 1, GCALL_TILES * TILE], BF, tag="g")
                gi_sb = gip.tile([128, GCALL_TILES * TILE // 16], I16, tag="gi")
                nc.sync.dma_start(
                    gi_sb[:, :nidx // 16],
                    gi_d[:, t0 * TILE // 16:(t0 * TILE + nidx) // 16])
                if EN_GATHER: nc.gpsimd.dma_gather(
                    gt[:, :, :nidx],
                    x_d[ch * SHARD:(ch + 1) * SHARD, :],
                    gi_sb[:, :nidx // 16],
                    nidx, nidx, 128, elem_step=128, transpose=True,
                    single_packet=False,
                    queue_num=(1 if QSPLIT else gci % NQ),
                )
                for tt in range(ntl):
                    t = t0 + tt
                    r = tile_meta[t][2]
                    w = t // 16
                    sl = t % 16
                    if sl == 0:
                        win_m[w] = mpool.tile([128, 16 * D_OUT], BF, tag="m", name=f"m{w}")
                    if t % 8 == 0:
                        banks[t] = mps.tile([128, 512], FP, tag="bank", name=f"bank{t}")
                    bank = banks[t - t % 8]
                    nc.tensor.matmul(
                        bank[:, (t % 8) * 64:(t % 8) * 64 + 64],
                        gt[0:D_IN, 0, tt * TILE:(tt + 1) * TILE],
                        w_all[0:64, r, :],
                        start=True, stop=True)
                    if t % 8 == 7:
                        h = (sl // 8)
                        dst = win_m[w][:, h * 512:(h + 1) * 512]
                        if (t // 8) % 2 == 0:
                            nc.scalar.copy(dst, bank[:])
                        else:
                            nc.vector.tensor_copy(dst, bank[:])
                        del banks[t - 7]
                    if sl == 15:
                        si_sb = sip.tile([128, WIN // 16], I16, tag="si")
                        nc.sync.dma_start(
                            si_sb[:], si_d[:, w * WIN // 16:(w + 1) * WIN // 16])
                        ae, ao = accs[(w + 1) % N_CHAINS]
                        if EN_SCATTER: nc.gpsimd.dma_scatter_add(
                            ae[:],
                            win_m[w][:].rearrange("p (b e) -> p b e", e=D_OUT),
                            si_sb[:], WIN, WIN, D_OUT,
                            sbuf_tokens_per_rank=128, parity_reg=0,
                            out_ap_other=ao[:],
                            single_packet=os.environ.get('K_SP', '0') == '1',
                            queue_num=(0 if QSPLIT else w % NQ),
                        )
                        del win_m[w]

            # ---- combine chains and write out ----
            CH = 14  # 98 = 7*14
            for par, out_d in ((0, oute_d), (1, outo_d)):
                for g0 in range(0, N_GRP, CH):
                    st = stp.tile([128, CH, D_OUT], FP, tag="st")
                    st2 = stp.tile([128, CH, D_OUT], FP, tag="st2")
                    sls = (slice(None), slice(g0, g0 + CH), slice(None))
                    nc.vector.tensor_add(st[:], accs[0][par][sls], accs[1][par][sls])
                    nc.vector.tensor_add(st2[:], accs[2][par][sls], accs[3][par][sls])
                    nc.vector.tensor_add(st[:], st[:], st2[:])
                    nc.sync.dma_start(out_d[:, g0:g0 + CH, :], st[:])

    nc.compile()
    return nc


# ---------------- top-level kernel ----------------

def kernel(x, node_keep_mask, source, target, edge_type, bases, att):
    x = np.asarray(x, np.float32)
    mask = np.asarray(node_keep_mask)
    bases = np.asarray(bases, np.float32)
    att = np.asarray(att, np.float32)

    import os
    import hashlib
    levels = gidx_all = sidx_all = None
    _h = hashlib.sha1()
    for _a in (source, target, edge_type):
        _h.update(np.ascontiguousarray(np.asarray(_a)).tobytes())
    _pc = f'/tmp/rgcn_plan_v1_{_h.hexdigest()[:12]}.npz'
    if os.environ.get('K_PLAN_CACHE', '1') == '1' and os.path.exists(_pc):
        try:
            _d = np.load(_pc)
            levels = [lv for lv in _d['levels']]
            gidx_all, sidx_all = _d['gidx'], _d['sidx']
        except Exception:
            levels = None
    if levels is None:
        levels, gidx_all, sidx_all = build_plan(
            np.asarray(source), np.asarray(target), np.asarray(edge_type))
        try:
            np.savez(_pc, levels=np.stack(levels), gidx=gidx_all, sidx=sidx_all)
        except Exception:
            pass
    S = gidx_all.shape[1]
    nc = build_nc(levels, S)

    x_pad = np.zeros((N_NODES, 128), ml_dtypes.bfloat16)
    x_pad[:, :D_IN] = x.astype(ml_dtypes.bfloat16)
    attT = np.ascontiguousarray(att.T)
    bas = np.ascontiguousarray(bases.reshape(NUM_BASES, -1))
    gi_w = wrap16(gidx_all)
    si_w = wrap16(sidx_all)

    in_maps = []
    for c in range(N_CORES):
        xs = np.zeros((SHARD_PAD, D_IN), np.float32)
        xs[:SHARD] = (x[c * SHARD:(c + 1) * SHARD]
                      * mask[c * SHARD:(c + 1) * SHARD]
                      .astype(np.float32)[:, None])
        xt = xs.T.astype(ml_dtypes.bfloat16)                    # [64, 25088]
        xt128 = np.ascontiguousarray(
            np.concatenate([xt[:, :HALF], xt[:, HALF:]], axis=0))  # [128, 12544]
        mk = np.zeros(SHARD_PAD, np.float32)
        mk[:SHARD] = mask[c * SHARD:(c + 1) * SHARD].astype(np.float32)
        mk = np.ascontiguousarray(mk.reshape(N_BLK, 128).T)     # [128, 196]
        in_maps.append({
            "x": x_pad, "xt": xt128, "mask": mk, "attT": attT, "bas": bas,
            "gi": gi_w[c], "si": si_w[c],
        })

    import os
    trace = os.environ.get("K_TRACE", "0") == "1"
    if trace:
        _install_ntff_shim()
    res = bass_utils.run_bass_kernel_spmd(
        nc, in_maps, core_ids=list(range(N_CORES)), trace=trace)
    if trace and res.exec_time_ns is not None:
        print(f"HW exec time: {res.exec_time_ns} ns", flush=True)
        kernel.last_exec_time_ns = res.exec_time_ns

    out = np.zeros((N_NODES, D_OUT), np.float32)
    v = np.arange(SHARD)
    sl, pt = v // 128, v % 128
    ev = (sl % 2) == 0
    for c in range(N_CORES):
        oe = res.results[c]["oute"]
        oo = res.results[c]["outo"]
        out[c * SHARD:(c + 1) * SHARD] = np.where(
            ev[:, None], oe[pt, sl // 2, :], oo[pt, sl // 2, :])
    return out

